# revision 21
# baseline (speedup 1.0000x reference)
# Trainium2 Bass kernel for AttentionBlock (conv-qkv + spatial softmax attention
# + 1x1 conv out + residual), data-parallel over batch on 8 NeuronCores.
#
# Math notes (per image, C=128 channels, N=64*64=4096 pixels):
#   q = conv3x3(x, Wq) + bq           [C, N]
#   k = conv3x3(x, Wk) + bk           [C, N]
#   v = conv3x3(x, Wv) + bv           [C, N]
#   A = softmax_j(q_i . k_j / sqrt(C))
#   out = x + Wo @ (A v) + bo
# Because Wo is 1x1 it commutes with the spatial mixing A, and because rows of
# A sum to 1 the bias bo commutes too:
#   Wo @ (A v) + bo = A @ (Wo v + bo) = A @ u,   u = conv3x3(x, Wo.Wv) + (Wo bv + bo)
# We compute attention unnormalized in the transposed layout
#   E[j, i] = exp(S^T[j, i] * scale + EXP_BIAS),  S^T = k^T q
#   O[c, i] = sum_j u[c, j] E[j, i]   (accumulated over j-tiles in PSUM)
#   colsum[i] = sum_j E[j, i]         (fp16 DVE accumulation + exact PE reduce)
#   out[c, i] = O[c, i] / colsum[i] + x[c, i]
# The EXP_BIAS and the softmax max-subtraction cancel in O/colsum; logits here
# are ~N(0,1) so no max subtraction is needed for fp stability.
#
# All matmul operands are fp16 (1 cycle/row on the PE; same 10-bit mantissa as
# tf32 for our value ranges); PSUM accumulation is fp32; the residual path and
# softmax denominators are fp32.

import numpy as np

try:
    import concourse.bass as bass  # noqa: F401
except ImportError:  # pragma: no cover
    import sys

    sys.path.insert(0, "/opt/trn_rl_repo")

import concourse.bass as bass
import concourse.mybir as mybir
from concourse import bacc
from concourse import tile
from concourse.masks import make_identity

B = 8
C = 128
H = W = 64
N = H * W  # 4096
HP = H + 2  # padded height/width 66
NTAP = 9
IB = 1024  # attention i-block (columns of the output per PSUM residency)
NIB = N // IB  # 4
NJT = N // 128  # 32 j-tiles
SCALE = float(C) ** -0.5
EXP_BIAS = -3.0

F32 = mybir.dt.float32
F16 = mybir.dt.float16

_CACHE = {}


def _build_nc():
    nc = bacc.Bacc(None)

    x_d = nc.dram_tensor("x", [C, H, W], F32, kind="ExternalInput")
    xh_d = nc.dram_tensor("xh", [C, HP, HP], F16, kind="ExternalInput")
    wq_d = nc.dram_tensor("wq", [C, NTAP, C], F16, kind="ExternalInput")
    wk_d = nc.dram_tensor("wk", [C, NTAP, C], F16, kind="ExternalInput")
    wu_d = nc.dram_tensor("wu", [C, NTAP, C], F16, kind="ExternalInput")
    bq_d = nc.dram_tensor("bq", [C, 1], F32, kind="ExternalInput")
    bk_d = nc.dram_tensor("bk", [C, 1], F32, kind="ExternalInput")
    bu_d = nc.dram_tensor("bu", [C, 1], F32, kind="ExternalInput")
    out_d = nc.dram_tensor("out", [C, H, W], F32, kind="ExternalOutput")

    with tile.TileContext(nc) as tc:
        with tc.tile_pool(name="persist", bufs=1) as pp:
            xpad = pp.tile([C, HP, HP], F16)  # fp16 zero-padded image (conv rhs)
            xres = pp.tile([C, N], F32)  # fp32 image for the residual add
            qb = pp.tile([C, N], F16)
            kb = pp.tile([C, N], F16)
            uT = pp.tile([C, NJT, 128], F16)  # u transposed: [j, (jt, c)]
            accs = pp.tile([C, N], F16)  # colsum partials per partition
            wq_s = pp.tile([C, NTAP, C], F16)
            wk_s = pp.tile([C, NTAP, C], F16)
            wu_s = pp.tile([C, NTAP, C], F16)
            bq_s = pp.tile([C, 1], F32)
            bk_s = pp.tile([C, 1], F32)
            bu_s = pp.tile([C, 1], F32)
            ebias = pp.tile([C, 1], F32)
            ident = pp.tile([128, 128], F16)
            ones = pp.tile([C, 1], F16)  # for the colsum reduction matmul
            csr = pp.tile([C, NIB, 8], F32)  # colsum redistributed 1024 -> [128,8]
            rcp = pp.tile([C, NIB, 8], F32)

            # DMA order matters: the u-conv runs first, so its operands lead.
            nc.sync.dma_start(xpad[:, 0:33, :], xh_d[:, 0:33, :])
            nc.sync.dma_start(wu_s, wu_d[:])
            nc.sync.dma_start(xpad[:, 33:HP, :], xh_d[:, 33:HP, :])
            nc.sync.dma_start(bu_s, bu_d[:])
            nc.sync.dma_start(wk_s, wk_d[:])
            nc.sync.dma_start(bk_s, bk_d[:])
            nc.sync.dma_start(wq_s, wq_d[:])
            nc.sync.dma_start(bq_s, bq_d[:])
            nc.sync.dma_start(xres.rearrange("p (a b) -> p a b", b=W), x_d[:])
            nc.vector.memset(ebias, EXP_BIAS)
            nc.vector.memset(ones, 1.0)
            make_identity(nc, ident)

            def conv_tile(pool, w_s, b_s, dest, t, tag="conv"):
                # output tile t covers rows [t*8, t*8+8) of the 64x64 image
                ps = pool.tile([C, 512], F32, tag=tag, name="ps")
                for tap in range(NTAP):
                    dy, dx = divmod(tap, 3)
                    rhs = xpad[:, t * 8 + dy : t * 8 + dy + 8, dx : dx + W]
                    nc.tensor.matmul(
                        ps,
                        w_s[:, tap, :],
                        rhs,
                        start=(tap == 0),
                        stop=(tap == NTAP - 1),
                    )
                nc.vector.tensor_scalar_add(dest[:, t * 512 : (t + 1) * 512], ps, b_s)

            ev = tc.alloc_tile_pool(name="ev", bufs=2)
            cps = tc.alloc_tile_pool(name="cps", bufs=1, space="PSUM")
            tps = tc.alloc_tile_pool(name="tps", bufs=1, space="PSUM")

            def uconv_tile(t):
                # u-conv tile: conv, add bias, transpose 128x128 blocks into
                # uT (fp16) via the PE transpose path.
                ps = cps.tile([C, 512], F32, tag="conv", name="ps")
                for tap in range(NTAP):
                    dy, dx = divmod(tap, 3)
                    rhs = xpad[:, t * 8 + dy : t * 8 + dy + 8, dx : dx + W]
                    nc.tensor.matmul(
                        ps, wu_s[:, tap, :], rhs,
                        start=(tap == 0), stop=(tap == NTAP - 1),
                    )
                ut = ev.tile([C, 512], F16, tag="utmp", name="ut")
                nc.vector.tensor_scalar_add(ut, ps, bu_s)
                for s in range(4):
                    tp = tps.tile([128, 128], F16, tag="tp", name="tp")
                    nc.tensor.transpose(tp, ut[:, s * 128 : (s + 1) * 128], ident)
                    nc.vector.tensor_copy(uT[:, t * 4 + s, :], tp)

            # Phase A: just enough convs to start the attention pipeline.
            for t in range(3):
                uconv_tile(t)
            for t in range(3):
                conv_tile(cps, wk_s, bk_s, kb, t)
            for t in range(2):
                conv_tile(cps, wq_s, bq_s, qb, t)

            # Remaining conv tiles stream inside the attention loops, one tile
            # per slot, placed between exp and the O-matmuls so the PE fills
            # its ACT-wait bubbles. (kind, tile) with deadlines inside ib0.
            ib_conv_sched = {
                0: {1: ("u", 3), 3: ("k", 3), 5: ("q", 2), 7: ("u", 4),
                    9: ("k", 4), 11: ("q", 3), 13: ("u", 5), 15: ("k", 5),
                    17: ("u", 6), 19: ("k", 6), 21: ("u", 7), 23: ("k", 7)},
                1: {5: ("q", 4), 15: ("q", 5)},
                2: {5: ("q", 6), 15: ("q", 7)},
                3: {},
            }

            # cps/tps stay alive through the attention phase so interleaved
            # convs never contend with the S-matmul PSUM slots.
            with tc.tile_pool(name="sps", bufs=2, space="PSUM") as sps, tc.tile_pool(
                name="ops", bufs=1, space="PSUM"
            ) as ops, tc.tile_pool(name="ep", bufs=4) as ep, tc.tile_pool(
                name="fin", bufs=2
            ) as fin, tc.tile_pool(name="dstage", bufs=1, space="DRAM") as dsp:
                rstage = dsp.tile([N], F32)  # reciprocal row bounced via DRAM
                for ib in range(NIB):
                    isl = slice(ib * IB, (ib + 1) * IB)
                    ob = ops.tile([C, IB], F32, tag="ob", name="ob")
                    for jt in range(NJT):
                        sp = sps.tile([C, IB], F32, tag="sp", name="sp")
                        for h in range(IB // 512):
                            nc.tensor.matmul(
                                sp[:, h * 512 : (h + 1) * 512],
                                kb[:, jt * 128 : (jt + 1) * 128],
                                qb[:, ib * IB + h * 512 : ib * IB + (h + 1) * 512],
                                start=True,
                                stop=True,
                            )
                        e = ep.tile([C, IB], F16, tag="e", name="e")
                        nc.scalar.activation(
                            e,
                            sp,
                            mybir.ActivationFunctionType.Exp,
                            bias=ebias,
                            scale=SCALE,
                        )
                        ins = ib_conv_sched[ib].get(jt)
                        if ins is not None:
                            kind, t = ins
                            if kind == "u":
                                uconv_tile(t)
                            elif kind == "k":
                                conv_tile(cps, wk_s, bk_s, kb, t)
                            else:
                                conv_tile(cps, wq_s, bq_s, qb, t)
                        for h in range(IB // 512):
                            nc.tensor.matmul(
                                ob[:, h * 512 : (h + 1) * 512],
                                uT[:, jt, :],
                                e[:, h * 512 : (h + 1) * 512],
                                start=(jt == 0),
                                stop=(jt == NJT - 1),
                            )
                        if jt == 0:
                            nc.vector.tensor_copy(accs[:, isl], e)
                        else:
                            nc.vector.tensor_add(accs[:, isl], accs[:, isl], e)

                    # colsum[i] = ones^T @ accs[:, isl] — exact f32 reduction on PE
                    csrow = fin.tile([1, IB], F32, tag="cs", name="csrow")
                    for h in range(IB // 512):
                        csrow_ps = cps.tile([1, 512], F32, tag="conv", name="csrow_ps")
                        nc.tensor.matmul(
                            csrow_ps,
                            ones,
                            accs[:, ib * IB + h * 512 : ib * IB + (h + 1) * 512],
                            start=True,
                            stop=True,
                        )
                        nc.vector.tensor_copy(csrow[:, h * 512 : (h + 1) * 512], csrow_ps)
                    # reciprocal with lane parallelism: [1,1024] -> [128,8]
                    nc.sync.dma_start(
                        csr[:, ib, :],
                        csrow[0:1, :].rearrange("p (a b) -> p a b", a=128),
                    )
                    nc.vector.reciprocal(rcp[:, ib, :], csr[:, ib, :])
                    nc.sync.dma_start(rstage[isl], rcp[:, ib, :])
                    rb = fin.tile([C, IB], F32, tag="rb", name="rb")
                    nc.sync.dma_start(rb, rstage[isl].partition_broadcast(C))
                    nt = fin.tile([C, IB], F32, tag="nt", name="nt")
                    nc.vector.tensor_mul(nt, ob, rb)
                    ot = fin.tile([C, IB], F32, tag="ot", name="ot")
                    nc.vector.tensor_add(ot, nt, xres[:, isl])
                    nc.sync.dma_start(out_d[:, ib * 16 : (ib + 1) * 16, :], ot)
            tps.release()
            cps.release()
            ev.release()

    nc.finalize()
    return nc


def get_nc():
    if "nc" not in _CACHE:
        _CACHE["nc"] = _build_nc()
    return _CACHE["nc"]


def _prep_host_inputs(x, Wq, bq, Wk, bk, Wv, bv, Wo, bo):
    x = np.ascontiguousarray(np.asarray(x, dtype=np.float32))
    Wq = np.asarray(Wq, dtype=np.float32)
    Wk = np.asarray(Wk, dtype=np.float32)
    Wv = np.asarray(Wv, dtype=np.float64)
    Wo2 = np.asarray(Wo, dtype=np.float64).reshape(C, C)
    bq = np.asarray(bq, dtype=np.float32)
    bk = np.asarray(bk, dtype=np.float32)
    bv = np.asarray(bv, dtype=np.float64)
    bo = np.asarray(bo, dtype=np.float64)

    # lhsT layouts: w[c, tap, o] = W[o, c, dy, dx]
    wq = np.ascontiguousarray(Wq.transpose(1, 2, 3, 0).reshape(C, NTAP, C))
    wk = np.ascontiguousarray(Wk.transpose(1, 2, 3, 0).reshape(C, NTAP, C))
    Wu = np.einsum("om,mckl->ockl", Wo2, Wv)
    wu = np.ascontiguousarray(Wu.transpose(1, 2, 3, 0).reshape(C, NTAP, C))
    bu = (Wo2 @ bv + bo).astype(np.float32)

    # fp16-padded image per core
    xpad = np.pad(x, ((0, 0), (0, 0), (1, 1), (1, 1))).astype(np.float16)

    shared = {
        "wq": wq.astype(np.float16),
        "wk": wk.astype(np.float16),
        "wu": wu.astype(np.float16),
        "bq": np.ascontiguousarray(bq.reshape(C, 1)),
        "bk": np.ascontiguousarray(bk.reshape(C, 1)),
        "bu": np.ascontiguousarray(bu.reshape(C, 1)),
    }
    in_maps = [
        dict(shared, x=np.ascontiguousarray(x[i]), xh=np.ascontiguousarray(xpad[i]))
        for i in range(B)
    ]
    return in_maps


def _run(inputs, trace=False):
    from concourse.bass_utils import run_bass_kernel_spmd

    in_maps = _prep_host_inputs(**inputs)
    nc = get_nc()
    res = run_bass_kernel_spmd(nc, in_maps, core_ids=list(range(B)), trace=trace)
    out = np.stack([np.asarray(res.results[i]["out"]) for i in range(B)])
    return out.reshape(B, C, H, W).astype(np.float32), res


def kernel(**inputs) -> np.ndarray:
    out, _ = _run(inputs, trace=False)
    return out


# revision 22
# speedup vs baseline: 1.0309x; 1.0309x over previous
# Trainium2 Bass kernel for AttentionBlock (conv-qkv + spatial softmax attention
# + 1x1 conv out + residual), data-parallel over batch on 8 NeuronCores.
#
# Math notes (per image, C=128 channels, N=64*64=4096 pixels):
#   q = conv3x3(x, Wq) + bq           [C, N]
#   k = conv3x3(x, Wk) + bk           [C, N]
#   v = conv3x3(x, Wv) + bv           [C, N]
#   A = softmax_j(q_i . k_j / sqrt(C))
#   out = x + Wo @ (A v) + bo
# Because Wo is 1x1 it commutes with the spatial mixing A, and because rows of
# A sum to 1 the bias bo commutes too:
#   Wo @ (A v) + bo = A @ (Wo v + bo) = A @ u,   u = conv3x3(x, Wo.Wv) + (Wo bv + bo)
# We compute attention unnormalized in the transposed layout
#   E[j, i] = exp(S^T[j, i] * scale + EXP_BIAS),  S^T = k^T q
#   O[c, i] = sum_j u[c, j] E[j, i]   (accumulated over j-tiles in PSUM)
#   colsum[i] = sum_j E[j, i]         (fp16 DVE accumulation + exact PE reduce)
#   out[c, i] = O[c, i] / colsum[i] + x[c, i]
# The EXP_BIAS and the softmax max-subtraction cancel in O/colsum; logits here
# are ~N(0,1) so no max subtraction is needed for fp stability.
#
# All matmul operands are fp16 (1 cycle/row on the PE; same 10-bit mantissa as
# tf32); PSUM accumulation is fp32; residual path and softmax denominators are
# fp32. The attention jt-loop is software-pipelined: S(jt+1) is emitted before
# O(jt) so the in-order PE stream keeps the ScalarE exp pipeline fed; q-conv
# taps for later i-blocks are threaded one-per-jt into the exp shadow.

import numpy as np

try:
    import concourse.bass as bass  # noqa: F401
except ImportError:  # pragma: no cover
    import sys

    sys.path.insert(0, "/opt/trn_rl_repo")

import concourse.bass as bass
import concourse.mybir as mybir
from concourse import bacc
from concourse import tile
from concourse.masks import make_identity

B = 8
C = 128
H = W = 64
N = H * W  # 4096
HP = H + 2  # padded height/width 66
NTAP = 9
IB = 1024  # attention i-block (columns of the output per PSUM residency)
NIB = N // IB  # 4
NJT = N // 128  # 32 j-tiles
SCALE = float(C) ** -0.5
EXP_BIAS = -3.0

F32 = mybir.dt.float32
F16 = mybir.dt.float16

_CACHE = {}


def _build_nc():
    nc = bacc.Bacc(None)

    x_d = nc.dram_tensor("x", [C, H, W], F32, kind="ExternalInput")
    xh_d = nc.dram_tensor("xh", [C, HP, HP], F16, kind="ExternalInput")
    wq_d = nc.dram_tensor("wq", [C, NTAP, C], F16, kind="ExternalInput")
    wk_d = nc.dram_tensor("wk", [C, NTAP, C], F16, kind="ExternalInput")
    wu_d = nc.dram_tensor("wu", [C, NTAP, C], F16, kind="ExternalInput")
    bq_d = nc.dram_tensor("bq", [C, 1], F32, kind="ExternalInput")
    bk_d = nc.dram_tensor("bk", [C, 1], F32, kind="ExternalInput")
    bu_d = nc.dram_tensor("bu", [C, 1], F32, kind="ExternalInput")
    out_d = nc.dram_tensor("out", [C, H, W], F32, kind="ExternalOutput")

    with tile.TileContext(nc) as tc:
        with tc.tile_pool(name="persist", bufs=1) as pp:
            xpad = pp.tile([C, HP, HP], F16)  # fp16 zero-padded image (conv rhs)
            xres = pp.tile([C, N], F32)  # fp32 image for the residual add
            qb = pp.tile([C, N], F16)
            kb = pp.tile([C, N], F16)
            uT = pp.tile([C, NJT, 128], F16)  # u transposed: [j, (jt, c)]
            accs = pp.tile([C, N], F16)  # colsum partials per partition
            wq_s = pp.tile([C, NTAP, C], F16)
            wk_s = pp.tile([C, NTAP, C], F16)
            wu_s = pp.tile([C, NTAP, C], F16)
            bq_s = pp.tile([C, 1], F32)
            bk_s = pp.tile([C, 1], F32)
            bu_s = pp.tile([C, 1], F32)
            ebias = pp.tile([C, 1], F32)
            ident = pp.tile([128, 128], F16)
            ones = pp.tile([C, 1], F16)  # for the colsum reduction matmul
            csr = pp.tile([C, NIB, 8], F32)  # colsum redistributed 1024 -> [128,8]
            rcp = pp.tile([C, NIB, 8], F32)

            # DMA order matters: the u-conv runs first, so its operands lead.
            nc.sync.dma_start(xpad[:, 0:33, :], xh_d[:, 0:33, :])
            nc.sync.dma_start(wu_s, wu_d[:])
            nc.sync.dma_start(bu_s, bu_d[:])
            nc.sync.dma_start(xpad[:, 33:HP, :], xh_d[:, 33:HP, :])
            nc.sync.dma_start(wk_s, wk_d[:])
            nc.sync.dma_start(bk_s, bk_d[:])
            nc.sync.dma_start(wq_s, wq_d[:])
            nc.sync.dma_start(bq_s, bq_d[:])
            nc.sync.dma_start(xres.rearrange("p (a b) -> p a b", b=W), x_d[:])
            nc.vector.memset(ebias, EXP_BIAS)
            nc.vector.memset(ones, 1.0)
            make_identity(nc, ident)

            ev = tc.alloc_tile_pool(name="ev", bufs=2)
            cps = tc.alloc_tile_pool(name="cps", bufs=2, space="PSUM")
            tps = tc.alloc_tile_pool(name="tps", bufs=2, space="PSUM")

            def conv_tap(ps, w_s, t, tap):
                dy, dx = divmod(tap, 3)
                rhs = xpad[:, t * 8 + dy : t * 8 + dy + 8, dx : dx + W]
                nc.tensor.matmul(
                    ps, w_s[:, tap, :], rhs,
                    start=(tap == 0), stop=(tap == NTAP - 1),
                )

            def conv_tile(w_s, b_s, dest, t):
                # output tile t covers rows [t*8, t*8+8) of the 64x64 image
                ps = cps.tile([C, 512], F32, tag="conv", name="ps")
                for tap in range(NTAP):
                    conv_tap(ps, w_s, t, tap)
                nc.vector.tensor_scalar_add(dest[:, t * 512 : (t + 1) * 512], ps, b_s)

            def uconv_tile(t):
                # u-conv tile: conv, add bias, transpose 128x128 blocks into
                # uT (fp16) via the PE transpose path.
                ps = cps.tile([C, 512], F32, tag="conv", name="ps")
                for tap in range(NTAP):
                    conv_tap(ps, wu_s, t, tap)
                ut = ev.tile([C, 512], F16, tag="utmp", name="ut")
                nc.vector.tensor_scalar_add(ut, ps, bu_s)
                for s in range(4):
                    tp = tps.tile([128, 128], F16, tag="tp", name="tp")
                    nc.tensor.transpose(tp, ut[:, s * 128 : (s + 1) * 128], ident)
                    nc.vector.tensor_copy(uT[:, t * 4 + s, :], tp)

            # Phase A: u, k and the first two q tiles.
            for t in range(8):
                uconv_tile(t)
            for t in range(8):
                conv_tile(wk_s, bk_s, kb, t)
            for t in range(2):
                conv_tile(wq_s, bq_s, qb, t)
            tps.release()
            ev.release()

            # q-conv taps for i-blocks 1..3 are threaded into the attention
            # loop, one tap per jt, placed in the shadow of the exp.
            # schedule[ib][jt] = (q-tile, tap)
            qsched = {ib: {} for ib in range(NIB)}
            for ib in range(NIB - 1):
                taps = [(2 * ib + 2 + (k // NTAP), k % NTAP) for k in range(2 * NTAP)]
                for idx, tt in enumerate(taps):
                    qsched[ib][4 + idx] = tt

            qps = {}  # live q-conv psum tile, keyed by q-tile index

            def q_tap(t, tap):
                if tap == 0:
                    qps[t] = cps.tile([C, 512], F32, tag="conv", name="qps")
                conv_tap(qps[t], wq_s, t, tap)
                if tap == NTAP - 1:
                    nc.vector.tensor_scalar_add(
                        qb[:, t * 512 : (t + 1) * 512], qps.pop(t), bq_s
                    )

            with tc.tile_pool(name="sps", bufs=2, space="PSUM") as sps, tc.tile_pool(
                name="ops", bufs=1, space="PSUM"
            ) as ops, tc.tile_pool(name="ep", bufs=4) as ep, tc.tile_pool(
                name="fin", bufs=2
            ) as fin, tc.tile_pool(name="dstage", bufs=1, space="DRAM") as dsp:
                rstage = dsp.tile([N], F32)  # reciprocal row bounced via DRAM

                def s_mm(ib, jt):
                    sp = sps.tile([C, IB], F32, tag="sp", name="sp")
                    for h in range(IB // 512):
                        nc.tensor.matmul(
                            sp[:, h * 512 : (h + 1) * 512],
                            kb[:, jt * 128 : (jt + 1) * 128],
                            qb[:, ib * IB + h * 512 : ib * IB + (h + 1) * 512],
                            start=True,
                            stop=True,
                        )
                    return sp

                for ib in range(NIB):
                    isl = slice(ib * IB, (ib + 1) * IB)
                    ob = ops.tile([C, IB], F32, tag="ob", name="ob")
                    sp = s_mm(ib, 0)
                    for jt in range(NJT):
                        e = ep.tile([C, IB], F16, tag="e", name="e")
                        nc.scalar.activation(
                            e,
                            sp,
                            mybir.ActivationFunctionType.Exp,
                            bias=ebias,
                            scale=SCALE,
                        )
                        # next S ahead of this jt's O: keeps PE feeding ACT
                        if jt + 1 < NJT:
                            sp = s_mm(ib, jt + 1)
                        tt = qsched[ib].get(jt)
                        if tt is not None:
                            q_tap(*tt)
                        for h in range(IB // 512):
                            nc.tensor.matmul(
                                ob[:, h * 512 : (h + 1) * 512],
                                uT[:, jt, :],
                                e[:, h * 512 : (h + 1) * 512],
                                start=(jt == 0),
                                stop=(jt == NJT - 1),
                            )
                        if jt == 0:
                            nc.vector.tensor_copy(accs[:, isl], e)
                        else:
                            nc.vector.tensor_add(accs[:, isl], accs[:, isl], e)

                    # colsum[i] = ones^T @ accs[:, isl] — exact f32 reduction on PE
                    csrow = fin.tile([1, IB], F32, tag="cs", name="csrow")
                    for h in range(IB // 512):
                        csrow_ps = cps.tile([1, 512], F32, tag="conv", name="csrow_ps")
                        nc.tensor.matmul(
                            csrow_ps,
                            ones,
                            accs[:, ib * IB + h * 512 : ib * IB + (h + 1) * 512],
                            start=True,
                            stop=True,
                        )
                        nc.vector.tensor_copy(csrow[:, h * 512 : (h + 1) * 512], csrow_ps)
                    # reciprocal with lane parallelism: [1,1024] -> [128,8]
                    nc.sync.dma_start(
                        csr[:, ib, :],
                        csrow[0:1, :].rearrange("p (a b) -> p a b", a=128),
                    )
                    nc.vector.reciprocal(rcp[:, ib, :], csr[:, ib, :])
                    nc.sync.dma_start(rstage[isl], rcp[:, ib, :])
                    rb = fin.tile([C, IB], F32, tag="rb", name="rb")
                    nc.sync.dma_start(rb, rstage[isl].partition_broadcast(C))
                    nt = fin.tile([C, IB], F32, tag="nt", name="nt")
                    nc.vector.tensor_mul(nt, ob, rb)
                    ot = fin.tile([C, IB], F32, tag="ot", name="ot")
                    nc.vector.tensor_add(ot, nt, xres[:, isl])
                    nc.sync.dma_start(out_d[:, ib * 16 : (ib + 1) * 16, :], ot)
            cps.release()

    nc.finalize()
    return nc


def get_nc():
    if "nc" not in _CACHE:
        _CACHE["nc"] = _build_nc()
    return _CACHE["nc"]


def _prep_host_inputs(x, Wq, bq, Wk, bk, Wv, bv, Wo, bo):
    x = np.ascontiguousarray(np.asarray(x, dtype=np.float32))
    Wq = np.asarray(Wq, dtype=np.float32)
    Wk = np.asarray(Wk, dtype=np.float32)
    Wv = np.asarray(Wv, dtype=np.float64)
    Wo2 = np.asarray(Wo, dtype=np.float64).reshape(C, C)
    bq = np.asarray(bq, dtype=np.float32)
    bk = np.asarray(bk, dtype=np.float32)
    bv = np.asarray(bv, dtype=np.float64)
    bo = np.asarray(bo, dtype=np.float64)

    # lhsT layouts: w[c, tap, o] = W[o, c, dy, dx]
    wq = np.ascontiguousarray(Wq.transpose(1, 2, 3, 0).reshape(C, NTAP, C))
    wk = np.ascontiguousarray(Wk.transpose(1, 2, 3, 0).reshape(C, NTAP, C))
    Wu = np.einsum("om,mckl->ockl", Wo2, Wv)
    wu = np.ascontiguousarray(Wu.transpose(1, 2, 3, 0).reshape(C, NTAP, C))
    bu = (Wo2 @ bv + bo).astype(np.float32)

    # fp16-padded image per core
    xpad = np.pad(x, ((0, 0), (0, 0), (1, 1), (1, 1))).astype(np.float16)

    shared = {
        "wq": wq.astype(np.float16),
        "wk": wk.astype(np.float16),
        "wu": wu.astype(np.float16),
        "bq": np.ascontiguousarray(bq.reshape(C, 1)),
        "bk": np.ascontiguousarray(bk.reshape(C, 1)),
        "bu": np.ascontiguousarray(bu.reshape(C, 1)),
    }
    in_maps = [
        dict(shared, x=np.ascontiguousarray(x[i]), xh=np.ascontiguousarray(xpad[i]))
        for i in range(B)
    ]
    return in_maps


def _run(inputs, trace=False):
    from concourse.bass_utils import run_bass_kernel_spmd

    in_maps = _prep_host_inputs(**inputs)
    nc = get_nc()
    res = run_bass_kernel_spmd(nc, in_maps, core_ids=list(range(B)), trace=trace)
    out = np.stack([np.asarray(res.results[i]["out"]) for i in range(B)])
    return out.reshape(B, C, H, W).astype(np.float32), res


def kernel(**inputs) -> np.ndarray:
    out, _ = _run(inputs, trace=False)
    return out


# revision 26
# speedup vs baseline: 1.1659x; 1.1309x over previous
# Trainium2 Bass kernel for AttentionBlock (conv-qkv + spatial softmax attention
# + 1x1 conv out + residual), data-parallel over batch on 8 NeuronCores.
#
# Math notes (per image, C=128 channels, N=64*64=4096 pixels):
#   q = conv3x3(x, Wq) + bq           [C, N]
#   k = conv3x3(x, Wk) + bk           [C, N]
#   v = conv3x3(x, Wv) + bv           [C, N]
#   A = softmax_j(q_i . k_j / sqrt(C))
#   out = x + Wo @ (A v) + bo
# Because Wo is 1x1 it commutes with the spatial mixing A, and because rows of
# A sum to 1 the bias bo commutes too:
#   Wo @ (A v) + bo = A @ (Wo v + bo) = A @ u,   u = conv3x3(x, Wo.Wv) + (Wo bv + bo)
# We compute attention unnormalized in the transposed layout
#   E[j, i] = exp(S^T[j, i] * scale + EXP_BIAS),  S^T = k^T q
#   O[c, i] = sum_j u[c, j] E[j, i]   (accumulated over j-tiles in PSUM)
#   colsum[i] = sum_j E[j, i]         (fp16 DVE accumulation + exact PE reduce)
#   out[c, i] = O[c, i] / colsum[i] + x[c, i]
# The EXP_BIAS and the softmax max-subtraction cancel in O/colsum; logits here
# are ~N(0,1) so no max subtraction is needed for fp stability.
#
# All matmul operands are fp16 (1 cycle/row on the PE; same 10-bit mantissa as
# tf32); PSUM accumulation is fp32; residual path and softmax denominators are
# fp32. The attention jt-loop is software-pipelined: S(jt+1) is emitted before
# O(jt) so the in-order PE stream keeps the ScalarE exp pipeline fed; q-conv
# taps for later i-blocks are threaded one-per-jt into the exp shadow.

import numpy as np

try:
    import concourse.bass as bass  # noqa: F401
except ImportError:  # pragma: no cover
    import sys

    sys.path.insert(0, "/opt/trn_rl_repo")

import concourse.bass as bass
import concourse.mybir as mybir
from concourse import bacc
from concourse import tile
from concourse.masks import make_identity

B = 8
C = 128
H = W = 64
N = H * W  # 4096
HP = H + 2  # padded height/width 66
NTAP = 9
IB = 1024  # attention i-block (columns of the output per PSUM residency)
NIB = N // IB  # 4
NJT = N // 128  # 32 j-tiles
SCALE = float(C) ** -0.5
EXP_BIAS = -3.0

F32 = mybir.dt.float32
F16 = mybir.dt.float16

_CACHE = {}


def _build_nc():
    nc = bacc.Bacc(None)

    x_d = nc.dram_tensor("x", [C, H, W], F32, kind="ExternalInput")
    xh_d = nc.dram_tensor("xh", [C, HP, HP], F16, kind="ExternalInput")
    wq_d = nc.dram_tensor("wq", [C, NTAP, C], F16, kind="ExternalInput")
    wk_d = nc.dram_tensor("wk", [C, NTAP, C], F16, kind="ExternalInput")
    wu_d = nc.dram_tensor("wu", [C, NTAP, C], F16, kind="ExternalInput")
    bq_d = nc.dram_tensor("bq", [C, 1], F32, kind="ExternalInput")
    bk_d = nc.dram_tensor("bk", [C, 1], F32, kind="ExternalInput")
    bu_d = nc.dram_tensor("bu", [C, 1], F32, kind="ExternalInput")
    out_d = nc.dram_tensor("out", [C, H, W], F32, kind="ExternalOutput")

    with tile.TileContext(nc) as tc:
        with tc.tile_pool(name="persist", bufs=1) as pp:
            xpad = pp.tile([C, HP, HP], F16)  # fp16 zero-padded image (conv rhs)
            xres = pp.tile([C, N], F32)  # fp32 image for the residual add
            qb = pp.tile([C, N], F16)
            kb = pp.tile([C, N], F16)
            uT = pp.tile([C, NJT, 128], F16)  # u transposed: [j, (jt, c)]
            accs = pp.tile([C, N], F16)  # colsum partials per partition
            wq_s = pp.tile([C, NTAP, C], F16)
            wk_s = pp.tile([C, NTAP, C], F16)
            wu_s = pp.tile([C, NTAP, C], F16)
            bq_s = pp.tile([C, 1], F32)
            bk_s = pp.tile([C, 1], F32)
            bu_s = pp.tile([C, 1], F32)
            ebias = pp.tile([C, 1], F32)
            ident = pp.tile([128, 128], F16)
            ones = pp.tile([C, 1], F16)  # for the colsum reduction matmul
            csr = pp.tile([C, NIB, 8], F32)  # colsum redistributed 1024 -> [128,8]
            rcp = pp.tile([C, NIB, 8], F32)

            # Input DMAs spread across HWDGE queues so they run in parallel;
            # the u-conv's operands (xpad, wu) lead.
            nc.sync.dma_start(xpad[:, 0:33, :], xh_d[:, 0:33, :])
            nc.scalar.dma_start(wu_s, wu_d[:])
            nc.scalar.dma_start(bu_s, bu_d[:])
            nc.sync.dma_start(xpad[:, 33:HP, :], xh_d[:, 33:HP, :])
            nc.scalar.dma_start(wk_s, wk_d[:])
            nc.scalar.dma_start(bk_s, bk_d[:])
            nc.sync.dma_start(wq_s, wq_d[:])
            nc.sync.dma_start(bq_s, bq_d[:])
            nc.scalar.dma_start(xres.rearrange("p (a b) -> p a b", b=W), x_d[:])
            nc.vector.memset(ebias, EXP_BIAS)
            nc.vector.memset(ones, 1.0)
            make_identity(nc, ident)

            ev = tc.alloc_tile_pool(name="ev", bufs=2)
            cps = tc.alloc_tile_pool(name="cps", bufs=2, space="PSUM")
            tps = tc.alloc_tile_pool(name="tps", bufs=2, space="PSUM")

            def conv_tap(ps, w_s, t, tap):
                dy, dx = divmod(tap, 3)
                rhs = xpad[:, t * 8 + dy : t * 8 + dy + 8, dx : dx + W]
                nc.tensor.matmul(
                    ps, w_s[:, tap, :], rhs,
                    start=(tap == 0), stop=(tap == NTAP - 1),
                )

            def conv_tile(w_s, b_s, dest, t):
                # output tile t covers rows [t*8, t*8+8) of the 64x64 image
                ps = cps.tile([C, 512], F32, tag="conv", name="ps")
                for tap in range(NTAP):
                    conv_tap(ps, w_s, t, tap)
                nc.vector.tensor_scalar_add(dest[:, t * 512 : (t + 1) * 512], ps, b_s)

            def uconv_tile(t):
                # u-conv tile: conv, add bias, transpose 128x128 blocks into
                # uT (fp16) via the PE transpose path.
                ps = cps.tile([C, 512], F32, tag="conv", name="ps")
                for tap in range(NTAP):
                    conv_tap(ps, wu_s, t, tap)
                ut = ev.tile([C, 512], F16, tag="utmp", name="ut")
                nc.vector.tensor_scalar_add(ut, ps, bu_s)
                for s in range(4):
                    tp = tps.tile([128, 128], F16, tag="tp", name="tp")
                    nc.tensor.transpose(tp, ut[:, s * 128 : (s + 1) * 128], ident)
                    nc.vector.tensor_copy(uT[:, t * 4 + s, :], tp)

            # Phase A: u, k and the first two q tiles.
            for t in range(8):
                uconv_tile(t)
            for t in range(8):
                conv_tile(wk_s, bk_s, kb, t)
            for t in range(2):
                conv_tile(wq_s, bq_s, qb, t)
            tps.release()
            ev.release()

            # q-conv taps for i-blocks 1..3 are threaded into the attention
            # loop, one tap per jt, placed in the shadow of the exp.
            # schedule[ib][jt] = (q-tile, tap)
            qsched = {ib: {} for ib in range(NIB)}
            for ib in range(NIB - 1):
                taps = [(2 * ib + 2 + (k // NTAP), k % NTAP) for k in range(2 * NTAP)]
                for idx, tt in enumerate(taps):
                    qsched[ib][4 + idx] = tt

            qps = {}  # live q-conv psum tile, keyed by q-tile index

            def q_tap(t, tap):
                if tap == 0:
                    qps[t] = cps.tile([C, 512], F32, tag="conv", name="qps")
                conv_tap(qps[t], wq_s, t, tap)
                if tap == NTAP - 1:
                    nc.vector.tensor_scalar_add(
                        qb[:, t * 512 : (t + 1) * 512], qps.pop(t), bq_s
                    )

            with tc.tile_pool(name="sps", bufs=2, space="PSUM") as sps, tc.tile_pool(
                name="ops", bufs=1, space="PSUM"
            ) as ops, tc.tile_pool(name="ep", bufs=4) as ep, tc.tile_pool(
                name="fin", bufs=2
            ) as fin, tc.tile_pool(name="dstage", bufs=1, space="DRAM") as dsp:
                rstage = dsp.tile([N], F32)  # reciprocal row bounced via DRAM

                def s_mm(ib, jt):
                    sp = sps.tile([C, IB], F32, tag="sp", name="sp")
                    for h in range(IB // 512):
                        nc.tensor.matmul(
                            sp[:, h * 512 : (h + 1) * 512],
                            kb[:, jt * 128 : (jt + 1) * 128],
                            qb[:, ib * IB + h * 512 : ib * IB + (h + 1) * 512],
                            start=True,
                            stop=True,
                        )
                    return sp

                sp = s_mm(0, 0)
                for ib in range(NIB):
                    isl = slice(ib * IB, (ib + 1) * IB)
                    ob = ops.tile([C, IB], F32, tag="ob", name="ob")
                    for jt in range(NJT):
                        e = ep.tile([C, IB], F16, tag="e", name="e")
                        nc.scalar.activation(
                            e,
                            sp,
                            mybir.ActivationFunctionType.Exp,
                            bias=ebias,
                            scale=SCALE,
                        )
                        # next S ahead of this jt's O (also across the ib
                        # boundary): keeps PE feeding ACT
                        nxt = ib * NJT + jt + 1
                        if nxt < NIB * NJT:
                            sp = s_mm(nxt // NJT, nxt % NJT)
                        tt = qsched[ib].get(jt)
                        if tt is not None:
                            q_tap(*tt)
                        for h in range(IB // 512):
                            nc.tensor.matmul(
                                ob[:, h * 512 : (h + 1) * 512],
                                uT[:, jt, :],
                                e[:, h * 512 : (h + 1) * 512],
                                start=(jt == 0),
                                stop=(jt == NJT - 1),
                            )
                        if jt == 0:
                            nc.vector.tensor_copy(accs[:, isl], e)
                        else:
                            nc.vector.tensor_add(accs[:, isl], accs[:, isl], e)

                    # Evict the O accumulator so the single ops slot frees
                    # immediately; normalization reads the SBUF copy.
                    obe = fin.tile([C, IB], F32, tag="obe", name="obe")
                    nc.vector.tensor_copy(obe, ob)

                    # colsum[i] = ones^T @ accs[:, isl] — exact f32 reduction on PE
                    csrow = fin.tile([1, IB], F32, tag="cs", name="csrow")
                    for h in range(IB // 512):
                        csrow_ps = cps.tile([1, 512], F32, tag="conv", name="csrow_ps")
                        nc.tensor.matmul(
                            csrow_ps,
                            ones,
                            accs[:, ib * IB + h * 512 : ib * IB + (h + 1) * 512],
                            start=True,
                            stop=True,
                        )
                        nc.vector.tensor_copy(csrow[:, h * 512 : (h + 1) * 512], csrow_ps)
                    # reciprocal with lane parallelism: [1,1024] -> [128,8]
                    nc.sync.dma_start(
                        csr[:, ib, :],
                        csrow[0:1, :].rearrange("p (a b) -> p a b", a=128),
                    )
                    nc.vector.reciprocal(rcp[:, ib, :], csr[:, ib, :])
                    nc.sync.dma_start(rstage[isl], rcp[:, ib, :])
                    rb = fin.tile([C, IB], F32, tag="rb", name="rb")
                    nc.sync.dma_start(rb, rstage[isl].partition_broadcast(C))
                    nt = fin.tile([C, IB], F32, tag="nt", name="nt")
                    nc.vector.tensor_mul(nt, obe, rb)
                    ot = fin.tile([C, IB], F32, tag="ot", name="ot")
                    nc.vector.tensor_add(ot, nt, xres[:, isl])
                    nc.sync.dma_start(out_d[:, ib * 16 : (ib + 1) * 16, :], ot)
            cps.release()

    nc.finalize()
    return nc


def get_nc():
    if "nc" not in _CACHE:
        _CACHE["nc"] = _build_nc()
    return _CACHE["nc"]


def _prep_host_inputs(x, Wq, bq, Wk, bk, Wv, bv, Wo, bo):
    x = np.ascontiguousarray(np.asarray(x, dtype=np.float32))
    Wq = np.asarray(Wq, dtype=np.float32)
    Wk = np.asarray(Wk, dtype=np.float32)
    Wv = np.asarray(Wv, dtype=np.float64)
    Wo2 = np.asarray(Wo, dtype=np.float64).reshape(C, C)
    bq = np.asarray(bq, dtype=np.float32)
    bk = np.asarray(bk, dtype=np.float32)
    bv = np.asarray(bv, dtype=np.float64)
    bo = np.asarray(bo, dtype=np.float64)

    # lhsT layouts: w[c, tap, o] = W[o, c, dy, dx]
    wq = np.ascontiguousarray(Wq.transpose(1, 2, 3, 0).reshape(C, NTAP, C))
    wk = np.ascontiguousarray(Wk.transpose(1, 2, 3, 0).reshape(C, NTAP, C))
    Wu = np.einsum("om,mckl->ockl", Wo2, Wv)
    wu = np.ascontiguousarray(Wu.transpose(1, 2, 3, 0).reshape(C, NTAP, C))
    bu = (Wo2 @ bv + bo).astype(np.float32)

    # fp16-padded image per core
    xpad = np.pad(x, ((0, 0), (0, 0), (1, 1), (1, 1))).astype(np.float16)

    shared = {
        "wq": wq.astype(np.float16),
        "wk": wk.astype(np.float16),
        "wu": wu.astype(np.float16),
        "bq": np.ascontiguousarray(bq.reshape(C, 1)),
        "bk": np.ascontiguousarray(bk.reshape(C, 1)),
        "bu": np.ascontiguousarray(bu.reshape(C, 1)),
    }
    in_maps = [
        dict(shared, x=np.ascontiguousarray(x[i]), xh=np.ascontiguousarray(xpad[i]))
        for i in range(B)
    ]
    return in_maps


def _run(inputs, trace=False):
    from concourse.bass_utils import run_bass_kernel_spmd

    in_maps = _prep_host_inputs(**inputs)
    nc = get_nc()
    res = run_bass_kernel_spmd(nc, in_maps, core_ids=list(range(B)), trace=trace)
    out = np.stack([np.asarray(res.results[i]["out"]) for i in range(B)])
    return out.reshape(B, C, H, W).astype(np.float32), res


def kernel(**inputs) -> np.ndarray:
    out, _ = _run(inputs, trace=False)
    return out
